# revision 3
# baseline (speedup 1.0000x reference)
"""GCN (2x GCNConv + linear head) on 8 NeuronCores via Bass/Tile.

v5 strategy (graph/data parallel, fp8 DoubleRow both layers):
  - Nodes padded to 10240 = 80 blocks of 128; core c owns dst range
    [c*1280, (c+1)*1280).
  - S = D^-1/2 (A+I) D^-1/2: A holds exact edge counts as dense fp8
    blocks; per-node scalings host-precomputed (g0 = dinv*x in fp8,
    dinvb broadcast tile).
  - Both aggregations run DoubleRow (paired 128-src-blocks, both
    operands fp8) at 2x PE rate; rel_err ~1.8e-2 vs the 2e-2 gate.
  - The first collective mesh cannot begin before a ~77us firmware
    floor, so everything local is free until then. v5 exploits it:
      * host permutes each core's A (and g0) so OWN src blocks sit at
        positions 0-9, PAIR-partner blocks at 10-19, the other 6 ranks
        rotated-pair order at 20-79 -- all L2 indices become static.
      * P0: own-block L2 accumulation runs pre-floor (free).
      * cores 2k/2k+1 share physical HBM: each writes its g1 slab to a
        pair-Shared DRAM tensor pre-floor; the 128-byte warm-up
        AllGather doubles as the pair barrier. P1: partner-block L2
        runs off the pair slab while the main AllGather mesh flies.
      * ONE main AllGather (fp8 packed in bf16 wire slots) carries the
        full slab; P2 (6 remote ranks, 20us PE) runs after it lands.
  - Manual dep edges are only ever instruction->instruction between
    ordinary DMAs/matmuls; deps ONTO collective_compute instructions
    crash the device (NRT_EXEC_UNIT_UNRECOVERABLE) -- ordering after a
    collective is done by dummy-reading its output tensor.
  - dinv_dst folded into the PSUM->SBUF cast; relu reads PSUM directly.
  - A streamed in consumption order across both HWDGE rings; epilogue
    (dinv mul, W2, relu, head, bias, store) staggered per dst chunk.
"""
import numpy as np
import ml_dtypes

import concourse.bass as bass
import concourse.mybir as mybir
import concourse.tile as tile
import concourse.bacc as bacc
from concourse.bass_utils import run_bass_kernel_spmd

FP8 = np.dtype(ml_dtypes.float8_e4m3)
BF16 = np.dtype(ml_dtypes.bfloat16)

N, E, D, C = 10000, 640000, 128, 40
NCORES = 8
NSB = 80                      # src blocks of 128
NPAD = NSB * 128              # 10240
DST = NPAD // NCORES          # 1280 dst nodes per core
CH = [(0, 512), (512, 512), (1024, 256)]
NHK = DST // 128              # head chunks of 128
WB = DST // 2                 # bf16 wire slots for the fp8 slab (640)
PAIR_HBM = True               # partner slab via pair-shared HBM (pre-AG)

_cache = {}


def _build():
    nc = bacc.Bacc("TRN2", target_bir_lowering=False, debug=False,
                   num_devices=NCORES)
    f32 = mybir.dt.float32
    bf16 = mybir.dt.bfloat16
    fp8 = mybir.dt.float8e4
    u32 = mybir.dt.uint32
    RELU = mybir.ActivationFunctionType.Relu
    DR = mybir.MatmulPerfMode.DoubleRow

    g0_d = nc.dram_tensor("g0", [128, NPAD], fp8, kind="ExternalInput")
    W1b = nc.dram_tensor("W1b", [D, D], bf16, kind="ExternalInput")
    W2b = nc.dram_tensor("W2b", [D, D], bf16, kind="ExternalInput")
    Wh = nc.dram_tensor("Wh", [D, C], f32, kind="ExternalInput")
    b1 = nc.dram_tensor("b1", [D, 1], f32, kind="ExternalInput")
    b2 = nc.dram_tensor("b2", [D, 1], f32, kind="ExternalInput")
    bhb_d = nc.dram_tensor("bhb", [128, C], f32, kind="ExternalInput")
    eye_d = nc.dram_tensor("eye", [128, 128], bf16, kind="ExternalInput")
    dinvb_d = nc.dram_tensor("dinvb", [128, DST], f32, kind="ExternalInput")
    ridx_d = nc.dram_tensor("ridx", [1, 8], u32, kind="ExternalInput")
    A_d = [nc.dram_tensor(f"A{ci}", [128, NSB * ln], fp8, kind="ExternalInput")
           for ci, (off, ln) in enumerate(CH)]
    out = nc.dram_tensor("out", [128, NHK * C], f32, kind="ExternalOutput")

    with tile.TileContext(nc) as tc:
        with (
            tc.tile_pool(name="big", bufs=1) as big,
            tc.tile_pool(name="sb", bufs=1) as sb,
            tc.tile_pool(name="tmpp", bufs=3) as tmpp,
            tc.tile_pool(name="psagg", bufs=3, space="PSUM") as psagg,
            tc.tile_pool(name="psz", bufs=1, space="PSUM") as psz,
            tc.tile_pool(name="pstr", bufs=2, space="PSUM") as pstr,
            tc.tile_pool(name="pshd", bufs=1, space="PSUM") as pshd,
            tc.tile_pool(name="dram", bufs=1, space="DRAM") as dram,
        ):
            # ---- all input streams issued up front, consumption order,
            # split across both HWDGE rings ----
            A_t = [big.tile([128, NSB * ln], fp8, name=f"At{ci}")
                   for ci, (off, ln) in enumerate(CH)]
            g0 = big.tile([128, NPAD], fp8)
            for p in range(4):
                s0, s1 = p * (NPAD // 4), (p + 1) * (NPAD // 4)
                (nc.sync if p % 2 == 0 else nc.scalar).dma_start(
                    g0[:, s0:s1], g0_d[:, s0:s1])
            dinvb = sb.tile([128, DST], f32)
            nc.scalar.dma_start(dinvb[:], dinvb_d[:, :])
            ridx_sb = sb.tile([1, 8], u32)
            nc.sync.dma_start(ridx_sb[:], ridx_d[:, :])
            W1_t = sb.tile([D, D], bf16)
            nc.scalar.dma_start(W1_t[:], W1b[:, :])
            b1_t = sb.tile([D, 1], f32)
            nc.scalar.dma_start(b1_t[:], b1[:, :])
            eye_t = sb.tile([128, 128], bf16)
            nc.scalar.dma_start(eye_t[:], eye_d[:, :])

            def stream_A(ci, npc):
                ln = CH[ci][1]
                for q in range(npc):
                    s0 = q * (NSB // npc) * ln
                    s1 = (q + 1) * (NSB // npc) * ln
                    (nc.sync if q % 2 == 0 else nc.scalar).dma_start(
                        A_t[ci][:, s0:s1], A_d[ci][:, s0:s1])

            stream_A(0, 8)
            stream_A(1, 4)
            stream_A(2, 4)
            W2_t = sb.tile([D, D], bf16)
            nc.scalar.dma_start(W2_t[:], W2b[:, :])
            b2_t = sb.tile([D, 1], f32)
            nc.scalar.dma_start(b2_t[:], b2[:, :])
            Wh_t = sb.tile([D, C], f32)
            nc.scalar.dma_start(Wh_t[:], Wh[:, :])
            bhb = sb.tile([128, C], f32)
            nc.scalar.dma_start(bhb[:], bhb_d[:, :])

            # per-core rank indices (loaded per engine that uses them)
            def load_sv(eng, col, maxv):
                r = eng.alloc_register(f"ridx{col}_{eng.engine.value}")
                eng.reg_load(r, ridx_sb[0:1, col:col + 1])
                return eng.snap(r, donate=True, min_val=0, max_val=maxv)

            sv_my = load_sv(nc.sync, 0, 1)      # my half of the pair tensor
            sv_pa = load_sv(nc.sync, 1, 1)      # partner half
            sv_sl = [load_sv(nc.sync if i % 2 == 0 else nc.scalar,
                             2 + i, NCORES - 1) for i in range(6)]

            # ---- layer 1 per chunk -> transpose -> node-major fp8 g1 ----
            cc_src = sb.tile([128, DST], fp8)
            last_tr = [None]

            def l1_chunk(ci):
                off, ln = CH[ci]
                agg = psagg.tile([128, 512], f32, tag="agg", name=f"agg1_{ci}")
                for pb in range(NSB // 2):
                    nc.tensor.matmul(
                        agg[:, :ln],
                        lhsT=g0[:, pb * 256:(pb + 1) * 256].rearrange(
                            "p (two f) -> p two f", two=2),
                        rhs=A_t[ci][:, pb * 2 * ln:(pb * 2 + 2) * ln].rearrange(
                            "p (two n) -> p two n", two=2),
                        start=(pb == 0), stop=(pb == NSB // 2 - 1),
                        perf_mode=DR)
                acc = tmpp.tile([128, 512], bf16, tag="acc")
                nc.vector.tensor_mul(acc[:, :ln], agg[:, :ln],
                                     dinvb[:, off:off + ln])
                zps = psz.tile([128, 512], f32, tag="z")
                nc.tensor.matmul(zps[:, :ln], lhsT=W1_t[:], rhs=acc[:, :ln],
                                 start=True, stop=True)
                t2 = tmpp.tile([128, 512], f32, tag="t2")
                nc.scalar.activation(t2[:, :ln], zps[:, :ln], RELU,
                                     bias=b1_t[:, 0:1], scale=1.0)
                g1c = tmpp.tile([128, 512], bf16, tag="g1c")
                nc.vector.tensor_mul(g1c[:, :ln], t2[:, :ln],
                                     dinvb[:, off:off + ln])
                nt = ln // 128
                trp = pstr.tile([128, 4, 128], bf16, tag="tr")
                for t in range(nt):
                    last_tr[0] = nc.tensor.transpose(
                        trp[:, t, :], g1c[:, t * 128:(t + 1) * 128], eye_t[:])
                nc.vector.tensor_copy(
                    cc_src[:, off:off + ln].rearrange(
                        "p (t f) -> p t f", f=128),
                    trp[:, :nt, :])

            l1_chunk(0)
            l1_chunk(1)
            l1_chunk(2)

            # ---- pair slab write + warm-up barrier (also absorbs the
            # firmware floor + cross-core alignment) ----
            T_pair = dram.tile([2, 128, WB], bf16, addr_space="Shared",
                               name="T_pair")
            warm_in = dram.tile([1, 128], bf16)
            warm_out = dram.tile([NCORES, 1, 128], bf16, addr_space="Shared")
            nc.sync.dma_start(T_pair[bass.ts(sv_my, 1), :, :],
                              cc_src[:, :].bitcast(bf16))
            # readback own half -> barrier input: orders barrier after write
            nc.sync.dma_start(warm_in[:],
                              T_pair[bass.ts(sv_my, 1), 0:1, 0:128])
            nc.gpsimd.collective_compute(
                "AllGather", mybir.AluOpType.bypass,
                replica_groups=[list(range(NCORES))],
                ins=[warm_in[:]], outs=[warm_out[:]])

            # ---- P0: own-src-block L2 accumulation (pre-floor, free) ----
            agg2 = [psagg.tile([128, 512], f32, tag="agg", name=f"agg2_{c2}")
                    for c2 in range(len(CH))]

            def l2_pairs(lhs_fp8, base_blk, p0, p1, chunks, start_k=None):
                first = None
                last = None
                for p in range(p0, p1):
                    lhsT = lhs_fp8[:, p * 256:(p + 1) * 256].rearrange(
                        "p (two f) -> p two f", two=2)
                    sb_g = base_blk + 2 * p
                    for c2 in chunks:
                        off2, ln2 = CH[c2]
                        mm = nc.tensor.matmul(
                            agg2[c2][:, :ln2], lhsT=lhsT,
                            rhs=A_t[c2][:, sb_g * ln2:(sb_g + 2) * ln2]
                            .rearrange("p (two n) -> p two n", two=2),
                            start=(start_k is not None and p == p0),
                            stop=False, perf_mode=DR)
                        if first is None:
                            first = mm
                        last = mm
                return first, last

            _, p0_last = l2_pairs(cc_src, 0, 0, 5, [0, 1, 2], start_k=True)

            # ---- after the barrier: dummy-read warm_out, then partner slab
            # (pair HBM) and the main AllGather input ----
            wo_sb = sb.tile([1, 128], bf16)
            dummy = nc.sync.dma_start(wo_sb[:], warm_out[0:1, 0:1, :])

            cc_in = dram.tile([128, WB], bf16, name="cc_in")
            cc_out = dram.tile([NCORES, 128, WB], bf16, addr_space="Shared",
                               name="cc_out")
            ccw = nc.scalar.dma_start(cc_in[:], cc_src[:, :].bitcast(bf16))
            bass._add_dep_helper(ccw.ins, dummy.ins, sync=True,
                                 reason="main AG input after warm-up barrier")
            nc.gpsimd.collective_compute(
                "AllGather", mybir.AluOpType.bypass,
                replica_groups=[list(range(NCORES))],
                ins=[cc_in[:]], outs=[cc_out[:]])

            # P1: partner blocks
            if PAIR_HBM:
                PT = sb.tile([128, WB], bf16, name="PT")
                ptd = nc.sync.dma_start(PT[:], T_pair[bass.ts(sv_pa, 1), :, :])
                bass._add_dep_helper(ptd.ins, dummy.ins, sync=True,
                                     reason="partner read after barrier")
                p1_first, p1_last = l2_pairs(PT[:].bitcast(fp8), 10, 0, 5,
                                             [0, 1, 2])
                bass._add_dep_helper(p1_first.ins, p0_last.ins, sync=True,
                                     reason="PE order: P1 after P0")
            else:
                p1_last = p0_last

            # ---- slab reads (6 remote ranks, dynamic slot indices) ----
            slabs = []
            for i in range(6):
                s = sb.tile([128, WB], bf16, name=f"slab{i}")
                (nc.sync if i % 2 == 0 else nc.scalar).dma_start(
                    s[:], cc_out[bass.ts(sv_sl[i], 1), :, :])
                slabs.append(s)
            if not PAIR_HBM:
                PT = sb.tile([128, WB], bf16, name="PT")
                ptd = nc.sync.dma_start(PT[:], cc_out[bass.ts(sv_pa, 1), :, :])
                p1_first, p1_last = l2_pairs(PT[:].bitcast(fp8), 10, 0, 5,
                                             [0, 1, 2])
                bass._add_dep_helper(p1_first.ins, p0_last.ins, sync=True,
                                     reason="PE order: P1 after P0")

            # ---- P2 chunk-major + staggered epilogue ----
            h2 = sb.tile([128, DST], f32)
            hd = pshd.tile([128, NHK * C], f32)
            out_sb = sb.tile([128, NHK * C], f32)
            hk0 = [0, 4, 8, 10]
            prev = p1_last
            for c2, (off2, ln2) in enumerate(CH):
                for i in range(6):
                    lhs = slabs[i][:].bitcast(fp8)
                    for p in range(5):
                        lhsT = lhs[:, p * 256:(p + 1) * 256].rearrange(
                            "p (two f) -> p two f", two=2)
                        sb_g = 20 + 10 * i + 2 * p
                        mm = nc.tensor.matmul(
                            agg2[c2][:, :ln2], lhsT=lhsT,
                            rhs=A_t[c2][:, sb_g * ln2:(sb_g + 2) * ln2]
                            .rearrange("p (two n) -> p two n", two=2),
                            start=False,
                            stop=(i == 5 and p == 4), perf_mode=DR)
                        if c2 == 0 and i == 0 and p == 0:
                            bass._add_dep_helper(
                                mm.ins, prev.ins, sync=True,
                                reason="PE order: P2 after P1")
                # epilogue for chunk c2
                acc = tmpp.tile([128, 512], bf16, tag="acc")
                nc.vector.tensor_mul(acc[:, :ln2], agg2[c2][:, :ln2],
                                     dinvb[:, off2:off2 + ln2])
                zps = psz.tile([128, 512], f32, tag="z")
                nc.tensor.matmul(zps[:, :ln2], lhsT=W2_t[:], rhs=acc[:, :ln2],
                                 start=True, stop=True)
                nc.scalar.activation(h2[:, off2:off2 + ln2], zps[:, :ln2],
                                     RELU, bias=b2_t[:, 0:1], scale=1.0)
                for hk in range(hk0[c2], hk0[c2 + 1]):
                    nc.tensor.matmul(hd[:, hk * C:(hk + 1) * C],
                                     lhsT=h2[:, hk * 128:(hk + 1) * 128],
                                     rhs=Wh_t[:], start=True, stop=True)
                nhb = hk0[c2 + 1] - hk0[c2]
                sl = slice(hk0[c2] * C, hk0[c2 + 1] * C)
                nc.vector.tensor_add(
                    out_sb[:, sl].rearrange("p (t c) -> p t c", c=C),
                    hd[:, sl].rearrange("p (t c) -> p t c", c=C),
                    bhb[:].unsqueeze(1).broadcast_to([128, nhb, C]))
                (nc.scalar if c2 % 2 == 0 else nc.sync).dma_start(
                    out[:, sl], out_sb[:, sl])
    nc.compile()
    return nc


def _prep(x, edge_index, W1, b1, W2, b2, Wh, bh):
    x = np.asarray(x, np.float32)
    ei = np.asarray(edge_index, np.int64)
    src = np.concatenate([ei[0], np.arange(NPAD, dtype=np.int64)])
    dst = np.concatenate([ei[1], np.arange(NPAD, dtype=np.int64)])
    deg = np.bincount(dst, minlength=NPAD).astype(np.float32)
    dinv = 1.0 / np.sqrt(deg)

    xp = np.zeros((NPAD, D), np.float32)
    xp[:N] = x
    g0 = dinv[:, None] * xp
    g0_nm = g0.reshape(NSB, 128, D).transpose(1, 0, 2).reshape(128, NPAD)
    g0_blocks = g0_nm.reshape(128, NSB, D)

    shared = {
        "W1b": np.asarray(W1, np.float32).astype(BF16),
        "W2b": np.asarray(W2, np.float32).astype(BF16),
        "Wh": np.asarray(Wh, np.float32),
        "b1": np.asarray(b1, np.float32).reshape(D, 1),
        "b2": np.asarray(b2, np.float32).reshape(D, 1),
        "bhb": np.broadcast_to(np.asarray(bh, np.float32).reshape(1, C),
                               (128, C)).copy(),
        "eye": np.eye(128, dtype=np.float32).astype(BF16),
    }
    core = dst // DST
    sl, sbk = src % 128, src // 128
    in_maps = []
    for c in range(NCORES):
        # rank->position map: own 0, partner 1, then rotated pairs
        rank_pos = {c: 0, c ^ 1: 1}
        rorder = []
        pr = c >> 1
        for j in range(1, 4):
            q = (pr + j) % 4
            rank_pos[2 * q] = 2 * j
            rank_pos[2 * q + 1] = 2 * j + 1
            rorder += [2 * q, 2 * q + 1]
        posb = np.array([rank_pos[b // NHK] * NHK + (b % NHK)
                         for b in range(NSB)], dtype=np.int64)
        # permute g0 blocks to match the per-core A block order
        g0_p = np.empty_like(g0_blocks)
        g0_p[:, posb, :] = g0_blocks
        ridx = np.array([[c & 1, (c & 1) ^ 1] + rorder], dtype=np.uint32)

        m = core == c
        dloc = dst[m] - c * DST
        im = dict(shared,
                  g0=np.ascontiguousarray(
                      g0_p.reshape(128, NPAD)).astype(FP8),
                  ridx=ridx,
                  dinvb=np.broadcast_to(
                      dinv[c * DST:(c + 1) * DST].reshape(1, DST),
                      (128, DST)).copy())
        for ci, (off, ln) in enumerate(CH):
            m2 = (dloc >= off) & (dloc < off + ln)
            Ac = np.zeros((128, NSB * ln), np.float32)
            np.add.at(Ac, (sl[m][m2],
                           posb[sbk[m][m2]] * ln + dloc[m2] - off), 1.0)
            im[f"A{ci}"] = Ac.astype(FP8)
        in_maps.append(im)
    return in_maps


def _run(inputs, trace=False):
    if "nc" not in _cache:
        _cache["nc"] = _build()
    in_maps = _prep(**inputs)
    res = run_bass_kernel_spmd(_cache["nc"], in_maps,
                               core_ids=list(range(NCORES)), trace=trace)
    # out is stored partition-major [128, NHK*C]; unpack to [DST, C]
    outs = []
    for c in range(NCORES):
        o = res.results[c]["out"].reshape(128, NHK, C)
        outs.append(o.transpose(1, 0, 2).reshape(DST, C))
    out = np.concatenate(outs, axis=0)[:N]
    return np.ascontiguousarray(out, dtype=np.float32), res


def kernel(**inputs):
    out, _ = _run(inputs, trace=False)
    return out


# revision 4
# speedup vs baseline: 1.8411x; 1.8411x over previous
"""GCN (2x GCNConv + linear head) on 8 NeuronCores via Bass/Tile.

v5 strategy (graph/data parallel, fp8 DoubleRow both layers):
  - Nodes padded to 10240 = 80 blocks of 128; core c owns dst range
    [c*1280, (c+1)*1280).
  - S = D^-1/2 (A+I) D^-1/2: A holds exact edge counts as dense fp8
    blocks; per-node scalings host-precomputed (g0 = dinv*x in fp8,
    dinvb broadcast tile).
  - Both aggregations run DoubleRow (paired 128-src-blocks, both
    operands fp8) at 2x PE rate; rel_err ~1.8e-2 vs the 2e-2 gate.
  - The first collective mesh cannot begin before a ~77us firmware
    floor, so everything local is free until then. v5 exploits it:
      * host permutes each core's A (and g0) so OWN src blocks sit at
        positions 0-9, PAIR-partner blocks at 10-19, the other 6 ranks
        rotated-pair order at 20-79 -- all L2 indices become static.
      * P0: own-block L2 accumulation runs pre-floor (free).
      * cores 2k/2k+1 share physical HBM: each writes its g1 slab to a
        pair-Shared DRAM tensor pre-floor; the 128-byte warm-up
        AllGather doubles as the pair barrier. P1: partner-block L2
        runs off the pair slab while the main AllGather mesh flies.
      * ONE main AllGather (fp8 packed in bf16 wire slots) carries the
        full slab; P2 (6 remote ranks, 20us PE) runs after it lands.
  - Manual dep edges are only ever instruction->instruction between
    ordinary DMAs/matmuls; deps ONTO collective_compute instructions
    crash the device (NRT_EXEC_UNIT_UNRECOVERABLE) -- ordering after a
    collective is done by dummy-reading its output tensor.
  - dinv_dst folded into the PSUM->SBUF cast; relu reads PSUM directly.
  - A streamed in consumption order across both HWDGE rings; epilogue
    (dinv mul, W2, relu, head, bias, store) staggered per dst chunk.
"""
import numpy as np
import ml_dtypes

import concourse.bass as bass
import concourse.mybir as mybir
import concourse.tile as tile
import concourse.bacc as bacc
from concourse.bass_utils import run_bass_kernel_spmd

FP8 = np.dtype(ml_dtypes.float8_e4m3)
BF16 = np.dtype(ml_dtypes.bfloat16)

N, E, D, C = 10000, 640000, 128, 40
NCORES = 8
NSB = 80                      # src blocks of 128
NPAD = NSB * 128              # 10240
DST = NPAD // NCORES          # 1280 dst nodes per core
CH = [(0, 512), (512, 512), (1024, 256)]
NHK = DST // 128              # head chunks of 128
WB = DST // 2                 # bf16 wire slots for the fp8 slab (640)

_cache = {}


def _build():
    nc = bacc.Bacc("TRN2", target_bir_lowering=False, debug=False,
                   num_devices=NCORES)
    f32 = mybir.dt.float32
    bf16 = mybir.dt.bfloat16
    fp8 = mybir.dt.float8e4
    u32 = mybir.dt.uint32
    RELU = mybir.ActivationFunctionType.Relu
    DR = mybir.MatmulPerfMode.DoubleRow

    g0_d = nc.dram_tensor("g0", [128, NPAD], fp8, kind="ExternalInput")
    W1b = nc.dram_tensor("W1b", [D, D], bf16, kind="ExternalInput")
    W2b = nc.dram_tensor("W2b", [D, D], bf16, kind="ExternalInput")
    Wh = nc.dram_tensor("Wh", [D, C], f32, kind="ExternalInput")
    b1 = nc.dram_tensor("b1", [D, 1], f32, kind="ExternalInput")
    b2 = nc.dram_tensor("b2", [D, 1], f32, kind="ExternalInput")
    bhb_d = nc.dram_tensor("bhb", [128, C], f32, kind="ExternalInput")
    eye_d = nc.dram_tensor("eye", [128, 128], bf16, kind="ExternalInput")
    dinvb_d = nc.dram_tensor("dinvb", [128, DST], f32, kind="ExternalInput")
    ridx_d = nc.dram_tensor("ridx", [1, 8], u32, kind="ExternalInput")
    A_d = [nc.dram_tensor(f"A{ci}", [128, NSB * ln], fp8, kind="ExternalInput")
           for ci, (off, ln) in enumerate(CH)]
    out = nc.dram_tensor("out", [128, NHK * C], f32, kind="ExternalOutput")

    with tile.TileContext(nc) as tc:
        with (
            tc.tile_pool(name="big", bufs=1) as big,
            tc.tile_pool(name="sb", bufs=1) as sb,
            tc.tile_pool(name="tmpp", bufs=3) as tmpp,
            tc.tile_pool(name="psagg", bufs=3, space="PSUM") as psagg,
            tc.tile_pool(name="psz", bufs=1, space="PSUM") as psz,
            tc.tile_pool(name="pstr", bufs=2, space="PSUM") as pstr,
            tc.tile_pool(name="pshd", bufs=1, space="PSUM") as pshd,
            tc.tile_pool(name="dram", bufs=1, space="DRAM") as dram,
        ):
            # ---- warm-up collective: the firmware floor is anchored at
            # the FIRST collective trigger (a late trigger delays the mesh
            # superlinearly -- measured: trigger@9us -> mesh@78us, but
            # trigger@61us -> mesh@184us), so it must fire with no data
            # deps, as early as possible ----
            warm_in = dram.tile([1, 128], bf16)
            warm_out = dram.tile([NCORES, 1, 128], bf16, addr_space="Shared")
            nc.gpsimd.collective_compute(
                "AllGather", mybir.AluOpType.bypass,
                replica_groups=[list(range(NCORES))],
                ins=[warm_in[:]], outs=[warm_out[:]])

            # ---- all input streams issued up front, consumption order,
            # split across both HWDGE rings ----
            A_t = [big.tile([128, NSB * ln], fp8, name=f"At{ci}")
                   for ci, (off, ln) in enumerate(CH)]
            g0 = big.tile([128, NPAD], fp8)
            for p in range(4):
                s0, s1 = p * (NPAD // 4), (p + 1) * (NPAD // 4)
                (nc.sync if p % 2 == 0 else nc.scalar).dma_start(
                    g0[:, s0:s1], g0_d[:, s0:s1])
            dinvb = sb.tile([128, DST], f32)
            nc.scalar.dma_start(dinvb[:], dinvb_d[:, :])
            ridx_sb = sb.tile([1, 8], u32)
            nc.sync.dma_start(ridx_sb[:], ridx_d[:, :])
            W1_t = sb.tile([D, D], bf16)
            nc.scalar.dma_start(W1_t[:], W1b[:, :])
            b1_t = sb.tile([D, 1], f32)
            nc.scalar.dma_start(b1_t[:], b1[:, :])
            eye_t = sb.tile([128, 128], bf16)
            nc.scalar.dma_start(eye_t[:], eye_d[:, :])

            def stream_A(ci, npc):
                ln = CH[ci][1]
                for q in range(npc):
                    s0 = q * (NSB // npc) * ln
                    s1 = (q + 1) * (NSB // npc) * ln
                    (nc.sync if q % 2 == 0 else nc.scalar).dma_start(
                        A_t[ci][:, s0:s1], A_d[ci][:, s0:s1])

            stream_A(0, 8)
            stream_A(1, 4)
            stream_A(2, 4)
            W2_t = sb.tile([D, D], bf16)
            nc.scalar.dma_start(W2_t[:], W2b[:, :])
            b2_t = sb.tile([D, 1], f32)
            nc.scalar.dma_start(b2_t[:], b2[:, :])
            Wh_t = sb.tile([D, C], f32)
            nc.scalar.dma_start(Wh_t[:], Wh[:, :])
            bhb = sb.tile([128, C], f32)
            nc.scalar.dma_start(bhb[:], bhb_d[:, :])

            # per-core rank indices (loaded per engine that uses them)
            def load_sv(eng, col, maxv):
                r = eng.alloc_register(f"ridx{col}_{eng.engine.value}")
                eng.reg_load(r, ridx_sb[0:1, col:col + 1])
                return eng.snap(r, donate=True, min_val=0, max_val=maxv)

            # slab slot ranks: col1=partner, cols2-7=rotated other pairs
            sv_sl = [load_sv(nc.sync if i % 2 == 0 else nc.scalar,
                             1 + i, NCORES - 1) for i in range(7)]

            # ---- layer 1 per chunk -> transpose -> node-major fp8 g1 ----
            cc_src = sb.tile([128, DST], fp8)
            last_tr = [None]

            def l1_chunk(ci):
                off, ln = CH[ci]
                agg = psagg.tile([128, 512], f32, tag="agg", name=f"agg1_{ci}")
                for pb in range(NSB // 2):
                    nc.tensor.matmul(
                        agg[:, :ln],
                        lhsT=g0[:, pb * 256:(pb + 1) * 256].rearrange(
                            "p (two f) -> p two f", two=2),
                        rhs=A_t[ci][:, pb * 2 * ln:(pb * 2 + 2) * ln].rearrange(
                            "p (two n) -> p two n", two=2),
                        start=(pb == 0), stop=(pb == NSB // 2 - 1),
                        perf_mode=DR)
                acc = tmpp.tile([128, 512], bf16, tag="acc")
                nc.vector.tensor_mul(acc[:, :ln], agg[:, :ln],
                                     dinvb[:, off:off + ln])
                zps = psz.tile([128, 512], f32, tag="z")
                nc.tensor.matmul(zps[:, :ln], lhsT=W1_t[:], rhs=acc[:, :ln],
                                 start=True, stop=True)
                t2 = tmpp.tile([128, 512], f32, tag="t2")
                nc.scalar.activation(t2[:, :ln], zps[:, :ln], RELU,
                                     bias=b1_t[:, 0:1], scale=1.0)
                g1c = tmpp.tile([128, 512], bf16, tag="g1c")
                nc.vector.tensor_mul(g1c[:, :ln], t2[:, :ln],
                                     dinvb[:, off:off + ln])
                nt = ln // 128
                trp = pstr.tile([128, 4, 128], bf16, tag="tr")
                for t in range(nt):
                    last_tr[0] = nc.tensor.transpose(
                        trp[:, t, :], g1c[:, t * 128:(t + 1) * 128], eye_t[:])
                nc.vector.tensor_copy(
                    cc_src[:, off:off + ln].rearrange(
                        "p (t f) -> p t f", f=128),
                    trp[:, :nt, :])

            l1_chunk(0)
            l1_chunk(1)
            l1_chunk(2)

            # ---- P0: own-src-block L2 accumulation (pre-floor, free) ----
            agg2 = [psagg.tile([128, 512], f32, tag="agg", name=f"agg2_{c2}")
                    for c2 in range(len(CH))]

            def l2_pairs(lhs_fp8, base_blk, p0, p1, chunks, start_k=None):
                first = None
                last = None
                for p in range(p0, p1):
                    lhsT = lhs_fp8[:, p * 256:(p + 1) * 256].rearrange(
                        "p (two f) -> p two f", two=2)
                    sb_g = base_blk + 2 * p
                    for c2 in chunks:
                        off2, ln2 = CH[c2]
                        mm = nc.tensor.matmul(
                            agg2[c2][:, :ln2], lhsT=lhsT,
                            rhs=A_t[c2][:, sb_g * ln2:(sb_g + 2) * ln2]
                            .rearrange("p (two n) -> p two n", two=2),
                            start=(start_k is not None and p == p0),
                            stop=False, perf_mode=DR)
                        if first is None:
                            first = mm
                        last = mm
                return first, last

            _, p0_last = l2_pairs(cc_src, 0, 0, 5, [0, 1, 2], start_k=True)

            # ---- main AllGather (fires after warm-up in CC FIFO order) ----
            cc_in = dram.tile([128, WB], bf16, name="cc_in")
            cc_out = dram.tile([NCORES, 128, WB], bf16, addr_space="Shared",
                               name="cc_out")
            nc.scalar.dma_start(cc_in[:], cc_src[:, :].bitcast(bf16))
            nc.gpsimd.collective_compute(
                "AllGather", mybir.AluOpType.bypass,
                replica_groups=[list(range(NCORES))],
                ins=[cc_in[:]], outs=[cc_out[:]])

            # ---- slab reads (partner + 6 remote ranks, dynamic slots) ----
            slabs = []
            for i in range(7):
                s = sb.tile([128, WB], bf16, name=f"slab{i}")
                (nc.sync if i % 2 == 0 else nc.scalar).dma_start(
                    s[:], cc_out[bass.ts(sv_sl[i], 1), :, :])
                slabs.append(s)
            p1_last = p0_last

            # ---- P2 chunk-major + staggered epilogue ----
            h2 = sb.tile([128, DST], f32)
            hd = pshd.tile([128, NHK * C], f32)
            out_sb = sb.tile([128, NHK * C], f32)
            hk0 = [0, 4, 8, 10]
            prev = p1_last
            for c2, (off2, ln2) in enumerate(CH):
                for i in range(7):
                    lhs = slabs[i][:].bitcast(fp8)
                    for p in range(5):
                        lhsT = lhs[:, p * 256:(p + 1) * 256].rearrange(
                            "p (two f) -> p two f", two=2)
                        sb_g = 10 + 10 * i + 2 * p
                        mm = nc.tensor.matmul(
                            agg2[c2][:, :ln2], lhsT=lhsT,
                            rhs=A_t[c2][:, sb_g * ln2:(sb_g + 2) * ln2]
                            .rearrange("p (two n) -> p two n", two=2),
                            start=False,
                            stop=(i == 6 and p == 4), perf_mode=DR)
                        if c2 == 0 and i == 0 and p == 0:
                            bass._add_dep_helper(
                                mm.ins, prev.ins, sync=True,
                                reason="PE order: P2 after P1")
                # epilogue for chunk c2
                acc = tmpp.tile([128, 512], bf16, tag="acc")
                nc.vector.tensor_mul(acc[:, :ln2], agg2[c2][:, :ln2],
                                     dinvb[:, off2:off2 + ln2])
                zps = psz.tile([128, 512], f32, tag="z")
                nc.tensor.matmul(zps[:, :ln2], lhsT=W2_t[:], rhs=acc[:, :ln2],
                                 start=True, stop=True)
                nc.scalar.activation(h2[:, off2:off2 + ln2], zps[:, :ln2],
                                     RELU, bias=b2_t[:, 0:1], scale=1.0)
                for hk in range(hk0[c2], hk0[c2 + 1]):
                    nc.tensor.matmul(hd[:, hk * C:(hk + 1) * C],
                                     lhsT=h2[:, hk * 128:(hk + 1) * 128],
                                     rhs=Wh_t[:], start=True, stop=True)
                nhb = hk0[c2 + 1] - hk0[c2]
                sl = slice(hk0[c2] * C, hk0[c2 + 1] * C)
                nc.vector.tensor_add(
                    out_sb[:, sl].rearrange("p (t c) -> p t c", c=C),
                    hd[:, sl].rearrange("p (t c) -> p t c", c=C),
                    bhb[:].unsqueeze(1).broadcast_to([128, nhb, C]))
                (nc.scalar if c2 % 2 == 0 else nc.sync).dma_start(
                    out[:, sl], out_sb[:, sl])
    nc.compile()
    return nc


def _prep(x, edge_index, W1, b1, W2, b2, Wh, bh):
    x = np.asarray(x, np.float32)
    ei = np.asarray(edge_index, np.int64)
    src = np.concatenate([ei[0], np.arange(NPAD, dtype=np.int64)])
    dst = np.concatenate([ei[1], np.arange(NPAD, dtype=np.int64)])
    deg = np.bincount(dst, minlength=NPAD).astype(np.float32)
    dinv = 1.0 / np.sqrt(deg)

    xp = np.zeros((NPAD, D), np.float32)
    xp[:N] = x
    g0 = dinv[:, None] * xp
    g0_nm = g0.reshape(NSB, 128, D).transpose(1, 0, 2).reshape(128, NPAD)
    g0_blocks = g0_nm.reshape(128, NSB, D)

    shared = {
        "W1b": np.asarray(W1, np.float32).astype(BF16),
        "W2b": np.asarray(W2, np.float32).astype(BF16),
        "Wh": np.asarray(Wh, np.float32),
        "b1": np.asarray(b1, np.float32).reshape(D, 1),
        "b2": np.asarray(b2, np.float32).reshape(D, 1),
        "bhb": np.broadcast_to(np.asarray(bh, np.float32).reshape(1, C),
                               (128, C)).copy(),
        "eye": np.eye(128, dtype=np.float32).astype(BF16),
    }
    core = dst // DST
    sl, sbk = src % 128, src // 128
    in_maps = []
    for c in range(NCORES):
        # rank->position map: own 0, partner 1, then rotated pairs
        rank_pos = {c: 0, c ^ 1: 1}
        rorder = []
        pr = c >> 1
        for j in range(1, 4):
            q = (pr + j) % 4
            rank_pos[2 * q] = 2 * j
            rank_pos[2 * q + 1] = 2 * j + 1
            rorder += [2 * q, 2 * q + 1]
        posb = np.array([rank_pos[b // NHK] * NHK + (b % NHK)
                         for b in range(NSB)], dtype=np.int64)
        # permute g0 blocks to match the per-core A block order
        g0_p = np.empty_like(g0_blocks)
        g0_p[:, posb, :] = g0_blocks
        ridx = np.array([[c & 1, c ^ 1] + rorder], dtype=np.uint32)

        m = core == c
        dloc = dst[m] - c * DST
        im = dict(shared,
                  g0=np.ascontiguousarray(
                      g0_p.reshape(128, NPAD)).astype(FP8),
                  ridx=ridx,
                  dinvb=np.broadcast_to(
                      dinv[c * DST:(c + 1) * DST].reshape(1, DST),
                      (128, DST)).copy())
        for ci, (off, ln) in enumerate(CH):
            m2 = (dloc >= off) & (dloc < off + ln)
            Ac = np.zeros((128, NSB * ln), np.float32)
            np.add.at(Ac, (sl[m][m2],
                           posb[sbk[m][m2]] * ln + dloc[m2] - off), 1.0)
            im[f"A{ci}"] = Ac.astype(FP8)
        in_maps.append(im)
    return in_maps


def _run(inputs, trace=False):
    if "nc" not in _cache:
        _cache["nc"] = _build()
    in_maps = _prep(**inputs)
    res = run_bass_kernel_spmd(_cache["nc"], in_maps,
                               core_ids=list(range(NCORES)), trace=trace)
    # out is stored partition-major [128, NHK*C]; unpack to [DST, C]
    outs = []
    for c in range(NCORES):
        o = res.results[c]["out"].reshape(128, NHK, C)
        outs.append(o.transpose(1, 0, 2).reshape(DST, C))
    out = np.concatenate(outs, axis=0)[:N]
    return np.ascontiguousarray(out, dtype=np.float32), res


def kernel(**inputs):
    out, _ = _run(inputs, trace=False)
    return out
